# revision 55
# baseline (speedup 1.0000x reference)
"""Multi-head attention kernel for Trainium2, sharded over 8 NeuronCores.

Problem: q,k,v [2, 4096, 256], 8 heads of d=32.  b*h = 16 head-instances
are sharded 2-per-core (core c -> batch c//4, head-pair c%4); no
cross-core communication.

Per-core algorithm (n=4096, d=32, 2 heads, fp16 operands / fp32 PSUM):

  phase 0: DMA q/k slabs [4096,64]; DVE casts to fp16 and duplicates the
    64 channels side-by-side -> staged [128,128] row-chunks; fp16
    PE-transpose (1 cyc/row) to kT/qTt [128, n] with the head-pair
    duplicated on partitions 64-127 (d rows: h0 @0-31, h1 @32-63, dup
    @64-127).  The duplicate feeds PE-array row-tiles (64,0)/(96,0).
    PSUM->SBUF copies alternate ScalarE/DVE.  V stored as per-k-chunk
    [128, 33] tiles with a ones-column appended (softmax denominator
    rides the PV matmul for free).

  main loop (q-tiles of 512, groups of 2 k-chunks of 128):
    - S^T for 2 heads x 2 k-chunks as FOUR concurrent 32-row PE tiles
      (tile_position (0,0),(32,0),(64,0),(96,0)) -> two PSUM groups
      [128, 2x512]; 100% PE row utilization (contraction d=32).
    - exp: split across TWO engines per a per-qt schedule:
        ScalarE chunks: one ACTIVATE Exp [128,1024] PSUM->SBUF fp16.
        DVE chunks: single-piece Schraudolph, ONE VectorE op:
          i = round(S * (scale*1024*log2e) + B)   int16  (1 elem/cyc)
        bitcast_fp16(i) = exp(S*scale) * (1 +- 3% sawtooth); the mean
        ratio is folded into B so DVE chunks match ScalarE chunks'
        scale; softmax normalization cancels any shared factor.
    - O^T[33,512] += [V|1]^T P per head, 2-way col-tiled PV matmuls
      ((0,0)/(0,64)) accumulated in one PSUM bank [97, 512]; row 32/96
      collects the denominator via the ones-column.  PV lags S by
      pv_skew=8 chunks so exp latency (esp. the DVE chain) never stalls
      the PE; the backlog drains during the next q-tile's groups.
    - per-q-tile epilogue (PE-transpose back, multiply by reciprocal
      denominator, DMA out) is spread across the next q-tile's groups.
"""

import numpy as np

import concourse.mybir as mybir
import concourse.tile as tile
from concourse import bacc, bass_utils
from concourse.masks import make_identity

_TAIL_FREE = 0       # 0: spread DVE chunks over all groups (a Scalar-only
                     # tail drains the 3-deep ps pool too slowly at the
                     # q-tile boundary and starves both exp engines)
_DVE_PER_QT = 12     # chunks per q-tile computed on the Vector engine
_PV_SKEW = 8         # chunks of S->exp lookahead before PV consumes

B, N, C, H, D = 2, 4096, 256, 8, 32
NCORES = 8
HPC = 2                      # heads per core
COLS = HPC * D               # 64 per-core channel columns
P = 128                      # partitions / k-chunk
QTILE = 512                  # q columns per head per PSUM group
NKC = N // P                 # 32 k-chunks
NQT = N // QTILE             # 8 q-tiles per head
NG = NKC // 2                # 16 k-chunk groups (4-way S tiling)
SCALE = float(1.0 / np.sqrt(D))
F32 = mybir.dt.float32
BF16 = mybir.dt.float16  # fp16: same PE speed as bf16, 4x finer mantissa
I16 = mybir.dt.int16

# Schraudolph pair-trick constants (see docstring).  i = round(A*x + Bc),
# P = fp16_bits(i) + 2^-0.5 * fp16_bits(i+512) approximates
# MEANR * exp(x*SCALE); log2(MEANR) is subtracted from the bias so the
# result matches ScalarE chunks' exp() scale.
LOG2E = float(1.0 / np.log(2.0))
AEXP = float(SCALE * 1024.0 * LOG2E)
MEANR = 2.0813747       # E[pair/exp] at zero bias shift; measured, mean->1
BEXP = float(1024.0 * 15.0 - 1024.0 * np.log2(MEANR))
HALF = float(2.0 ** -0.5)

_cache = {}


def _dve_sched(dve_per_qt, tail_free=_TAIL_FREE):
    """Bresenham spread of DVE-exp chunks over the first NG - tail_free
    groups of a q-tile (the final groups stay on ScalarE so it carries a
    backlog into the q-tile boundary while the PE drains PVs; DVE has
    per-tile slack, so its boundary idle is free).  Returns per-group
    (kcA_dve, kcB_dve); kcA prefers ScalarE."""
    ng = NG - tail_free
    sched = []
    for g in range(NG):
        if g >= ng:
            sched.append((False, False))
            continue
        n0 = (g * dve_per_qt) // ng
        n1 = ((g + 1) * dve_per_qt) // ng
        take = n1 - n0          # 0, 1 (or 2 when dve_per_qt > ng)
        sched.append((take >= 2, take >= 1))
    return sched


def _emit(tc, nc, q, k, v, out, heads=HPC, dve_per_qt=_DVE_PER_QT,
          do_s=True, do_exp=True, do_pv=True, do_main=True,
          pv_skew=_PV_SKEW):
    sched = _dve_sched(dve_per_qt)
    with tc.tile_pool(name="persist", bufs=1) as persist:
        ident = persist.tile([P, P], F32, name="ident")
        make_identity(nc, ident[:])
        # kT: xbar-transposed K with TWO k-chunks stacked per 128-col block:
        # block g partitions 0-63 = chunk 2g (h0 rows 0-31, h1 rows 32-63),
        # partitions 64-127 = chunk 2g+1.  Chunk c holds k rows {32p + c}
        # (a permutation of the natural order; V is gathered to match, and
        # softmax contraction is k-order invariant).
        kT = persist.tile([P, NG * P], BF16, name="kT")
        # qT duplicated layout: partitions 0-63 = (h0 d, h1 d), 64-127 =
        # copy (feeds PE row-tiles (64,0)/(96,0)); q contiguous on columns.
        qT = persist.tile([P, N], BF16, name="qT")
        # V with ones column: per head, 32 chunks of [128, 33].  vbs is
        # the same scaled by 2^-0.5 — the second Schraudolph pair piece's
        # weight, so the pair-sum rides the PV PSUM accumulation.
        vsb = persist.tile([P, HPC * NKC * (D + 1)], BF16, name="vsb")
        vbs = persist.tile([P, HPC * NKC * (D + 1)], BF16, name="vbs")

        identh = persist.tile([P, P], BF16, name="identh")
        nc.vector.tensor_copy(identh[:], ident[:])
        # K: two contiguous fp32 loads (cheap, 128 descriptors each),
        # DVE cast to fp16, xbar block-transpose per half straight into
        # the stacked-chunk kT layout.  Q: gathered per quarter
        # (n-on-partitions); its cast + duplicate + fp16 PE-transposes
        # are emitted LAZILY inside the main loop (just ahead of the
        # q-tiles that need them) so the per-engine FIFOs never park the
        # first S matmuls behind staging that waits on late gathers.
        # V: gathered in the chunk order matching kT's xbar permutation;
        # its SBUF staging is likewise emitted inside q-tile 0.
        NH = 2
        KHF = N * COLS // P // NH        # fp16 elems per partition/half
        NQ = 4
        CPQ = NKC // NQ                  # 8 q-chunks per quarter
        VHC = NKC // NH
        stage_pool = persist
        kf = [stage_pool.tile([P, KHF], F32, name=f"kf{h}")
              for h in range(NH)]
        kst = [stage_pool.tile([P, KHF], BF16, name=f"kst{h}")
               for h in range(NH)]
        qf = [stage_pool.tile([P, CPQ * COLS], F32, name=f"qf{g}")
              for g in range(NQ)]
        stq = [stage_pool.tile([P, CPQ * 2 * COLS], BF16,
                               name=f"stq{g}") for g in range(NQ)]
        vstage = stage_pool.tile([P, NKC * COLS], F32, name="vstage")
        vsv = vstage[:].rearrange("p (c d) -> p c d", d=COLS)
        onescol = persist.tile([P, HPC * NKC], F32, name="onescol")

        qsrc = q.rearrange("(i p) d -> i p d", p=P)

        def qload(g):
            nc.gpsimd.dma_start(
                qf[g][:].rearrange("p (i d) -> p i d", d=COLS),
                qsrc[g * CPQ:(g + 1) * CPQ].rearrange("i p d -> p i d"),
            )

        # Ring split: k halves + q quarter 0 (they gate the first S group)
        # on the HWDGE sync ring; v and q quarters 1-3 on the SWDGE
        # (gpsimd) ring, which drains concurrently — v is needed by the
        # first PV pops (~4 groups in) and must not queue behind q0.
        nc.gpsimd.dma_start(
            kf[0][:], k.rearrange("(h p x) d -> h p (x d)", h=NH, p=P)[0]
        )
        qload(0)
        nc.gpsimd.dma_start(
            kf[1][:], k.rearrange("(h p x) d -> h p (x d)", h=NH, p=P)[1]
        )
        for h in range(NH):
            nc.gpsimd.dma_start(
                vsv[:, h * VHC:(h + 1) * VHC, :],
                v.rearrange("(h p c) d -> h p c d", h=NH, p=P)[h],
            )
        for g in range(1, NQ):
            nc.gpsimd.dma_start(
                qf[g][:].rearrange("p (i d) -> p i d", d=COLS),
                qsrc[g * CPQ:(g + 1) * CPQ].rearrange("i p d -> p i d"),
            )

        # k casts on ScalarE (idle until the first exp; keeps the DVE FIFO
        # free for q quarter 0's cast+dup, which gates the first S group);
        # xbars at the tail of the sync HWDGE queue: FIFO puts them after
        # the three sync loads, and every DMA scheduled after them (only
        # the much-later output stores) inherits their wait.
        for h in range(NH):
            nc.vector.tensor_copy(kst[h][:], kf[h][:])
            nc.sync.dma_start_transpose(
                kT[:, h * KHF:(h + 1) * KHF].rearrange(
                    "p (b c) -> p b c", c=P),
                kst[h][:],
            )

        def stage_v():
            vv = vsb[:].rearrange("p (hh i e) -> p hh i e",
                                  hh=HPC, e=D + 1)
            vst = vstage[:].rearrange("p (i d) -> p i d", d=COLS)
            for hh in range(HPC):
                nc.vector.tensor_copy(
                    vv[:, hh, :, 0:D], vst[:, :, hh * D:(hh + 1) * D]
                )
            nc.vector.memset(onescol[:], 1.0)
            nc.vector.tensor_copy(
                vv[:, :, :, D],
                onescol[:].rearrange("p (hh i) -> p hh i", hh=HPC),
            )
            nc.vector.tensor_scalar_mul(vbs[:], vsb[:], HALF)

        if not do_main:
            return
        with (
            tc.tile_pool(name="ps", bufs=3, space="PSUM") as ps_pool,
            tc.tile_pool(name="po", bufs=2, space="PSUM") as po_pool,
            tc.tile_pool(name="pexp", bufs=pv_skew + 2) as pexp_pool,
            tc.tile_pool(name="pint", bufs=pv_skew // 2 + 2) as pint_pool,
            tc.tile_pool(name="pint2", bufs=pv_skew // 2 + 2) as pint2_pool,
            tc.tile_pool(name="osb", bufs=2) as osb_pool,
            tc.tile_pool(name="rec", bufs=3) as rec_pool,
            tc.tile_pool(name="outsb", bufs=3) as outsb_pool,
        ):
            if not do_s:
                ps_fix = ps_pool.tile([P, HPC * QTILE], F32, tag="ps")
                nc.vector.memset(ps_fix[:], 0.25)
            if not do_exp:
                pexp_fix = pexp_pool.tile([P, HPC * QTILE], BF16, tag="pexp")
                nc.vector.memset(pexp_fix[:], 0.5)

            def stage_quarter(g):
                """Cast + duplicate + PE-transpose q quarter g into qT
                (borrows ps_pool slots for the transpose PSUM tiles)."""
                sv = stq[g][:].rearrange("p (i two d) -> p i two d",
                                         two=2, d=COLS)
                qv = qf[g][:].rearrange("p (i d) -> p i d", d=COLS)
                nc.vector.tensor_copy(sv[:, :, 0, :], qv)
                nc.vector.tensor_copy(sv[:, :, 1, :], sv[:, :, 0, :])
                for j in range(CPQ // 4):
                    pt = ps_pool.tile([P, 4 * P], BF16, tag="ps",
                                      name=f"pt{g}_{j}", uniquify=True)
                    for jj in range(4):
                        ii = 4 * j + jj
                        nc.tensor.transpose(
                            pt[:, jj * P:(jj + 1) * P],
                            stq[g][:, ii * 2 * COLS:(ii + 1) * 2 * COLS],
                            identh[:],
                        )
                    dst = qT[:, (2 * g + j) * 512:(2 * g + j + 1) * 512]
                    nc.vector.tensor_copy(dst, pt[:])

            stage_quarter(0)

            def emit_s_half(qt, g, half):
                """One half of the 4-way row-tiled S^T group: half 0 ->
                chunk 2g on PE rows 0-63 (kT block g partitions 0-63),
                half 1 -> chunk 2g+1 on rows 64-127."""
                if not do_s:
                    return ps_fix
                ps = ps_pool.tile([P, HPC * QTILE], F32, tag="ps")
                base = 64 * half
                for hh in range(heads):
                    rp = slice(base + D * hh, base + D * (hh + 1))
                    nc.tensor.matmul(
                        ps[:, hh * QTILE:(hh + 1) * QTILE],
                        lhsT=kT[rp, g * P:(g + 1) * P],
                        rhs=qT[rp, qt * QTILE:(qt + 1) * QTILE],
                        start=True, stop=True,
                        tile_position=(base + D * hh, 0),
                    )
                return ps

            def emit_exp(ps, use_dve):
                """exp of one [128, 2*512] S chunk.  ScalarE: one fp16 tile.
                DVE: two int16 Schraudolph pieces (summed later on the PE).
                The fp16 S source gives the DVE op 2x_1P packed reads."""
                if not do_exp:
                    return ((pexp_fix,), False)
                if not use_dve:
                    pexp = pexp_pool.tile([P, HPC * QTILE], BF16, tag="pexp")
                    nc.scalar.activation(
                        pexp[:], ps[:], mybir.ActivationFunctionType.Exp,
                        scale=SCALE,
                    )
                    return ((pexp,), False)
                pint = pint_pool.tile([P, HPC * QTILE], I16, tag="pint")
                nc.vector.tensor_scalar(
                    pint[:], ps[:], AEXP, BEXP,
                    mybir.AluOpType.mult, mybir.AluOpType.add,
                )
                pint2 = pint2_pool.tile([P, HPC * QTILE], I16, tag="pint2")
                nc.vector.tensor_scalar_add(pint2[:], pint[:], 512)
                return ((pint, pint2), True)

            def emit_pv(poc, pieces, kc, first, last):
                if not do_pv:
                    return
                tiles, isint = pieces
                vws = (vsb, vbs)[:len(tiles)]
                for pi, (tl, vw) in enumerate(zip(tiles, vws)):
                    st = first and pi == 0
                    sp = last and pi == len(tiles) - 1
                    for hh in range(heads):
                        vbase = hh * NKC * (D + 1)
                        vch = vw[:, vbase + kc * (D + 1):
                                 vbase + (kc + 1) * (D + 1)]
                        rhs = tl[:, hh * QTILE:(hh + 1) * QTILE]
                        if isint:
                            rhs = rhs.bitcast(BF16)
                        nc.tensor.matmul(
                            poc[64 * hh:64 * hh + D + 1, :],
                            lhsT=vch,
                            rhs=rhs,
                            start=st,
                            stop=sp,
                            skip_group_check=True,
                            tile_position=(0, 64 * hh),
                        )

            def epilogue_steps(pocl, q0):
                def copy_step():
                    osb = osb_pool.tile([97, QTILE], F32, tag="osb",
                                        name="osb", uniquify=True)
                    osbs[0] = osb
                    for hh in range(heads):
                        ib = 64 * hh
                        nc.vector.tensor_copy(
                            osb[ib:ib + D + 1, :], pocl[ib:ib + D + 1, :]
                        )
                def norm_step(j):
                    # both heads -> one [128, 64] tile -> one contiguous
                    # DMA; both transposes share ONE ps-pool slot so the
                    # epilogue never starves S of PSUM buffers
                    outsb = outsb_pool.tile([P, COLS], F32, tag="outsb")
                    pt2 = ps_pool.tile([P, 2 * (D + 1)], F32, tag="ps",
                                       name="pt2", uniquify=True)
                    for hh in range(heads):
                        ib = 64 * hh
                        ob = (D + 1) * hh
                        nc.tensor.transpose(
                            pt2[:, ob:ob + D + 1],
                            osbs[0][ib:ib + D + 1, j * P:(j + 1) * P],
                            ident[ib:ib + D + 1, ib:ib + D + 1],
                        )
                        rec = rec_pool.tile([P, 1], F32, tag="rec")
                        nc.vector.reciprocal(
                            rec[:], pt2[:, ob + D:ob + D + 1]
                        )
                        nc.vector.tensor_scalar_mul(
                            outsb[:, D * hh:D * (hh + 1)],
                            pt2[:, ob:ob + D], rec[:],
                        )
                    nc.sync.dma_start(
                        out[q0 + j * P:q0 + (j + 1) * P, :], outsb[:]
                    )
                osbs = {}
                steps = [copy_step]
                for j in range(QTILE // P):
                    steps.append(lambda j=j: norm_step(j))
                return steps

            pending = []          # deferred epilogue of the previous q-tile
            pvq = []              # (pieces, kc, poc, q0) across q-tiles

            def pop_pv():
                pieces, kc, poc, q0 = pvq.pop(0)
                emit_pv(poc, pieces, kc, kc == 0, kc == NKC - 1)
                if kc == NKC - 1:
                    # this q-tile's accumulation is complete; queue its
                    # epilogue to be spread over the following groups
                    pending.extend(epilogue_steps(poc, q0))

            for qt in range(NQT):
                q0 = qt * QTILE
                if qt % 2 == 1 and qt < NQT - 1:
                    stage_quarter((qt + 1) // 2)
                poc = po_pool.tile([97, QTILE], F32, tag="po",
                                   name=f"po_{qt}")
                for g in range(NG):
                    if qt == 0 and g == 2:
                        stage_v()
                    psA = emit_s_half(qt, g, 0)
                    psB = emit_s_half(qt, g, 1)
                    dveA, dveB = sched[g]
                    pexpA = emit_exp(psA, dveA)
                    pexpB = emit_exp(psB, dveB)
                    pvq.append((pexpA, 2 * g, poc, q0))
                    pvq.append((pexpB, 2 * g + 1, poc, q0))
                    # taper the skew at the very end so the tail drains
                    # alongside the last groups instead of after them
                    thr = pv_skew
                    if qt == NQT - 1:
                        thr = min(pv_skew, 2 * (NG - 1 - g))
                    while len(pvq) > thr:
                        pop_pv()
                    # epilogue steps AFTER the PV pops so their transposes
                    # never wedge between a PV pair's two col-tiled MMs;
                    # one per 3 groups spreads them across the q-tile
                    if pending and g % 3 == 0 and g > 0:
                        pending.pop(0)()
            while pvq:
                pop_pv()
            for step in pending:
                step()


def _build(loop=0, **emit_kw):
    """loop=0: production build.  loop>=1: body wrapped in an on-device
    For_i repeat loop (timing-only builds).  emit_kw: ablation knobs."""
    key = ("nc", loop, tuple(sorted(emit_kw.items())))
    if key in _cache:
        return _cache[key]
    nc = bacc.Bacc(
        "TRN2",
        target_bir_lowering=False,
        debug=False,
        enable_asserts=False,
        num_devices=NCORES,
    )
    q = nc.dram_tensor("q", [N, COLS], F32, kind="ExternalInput").ap()
    k = nc.dram_tensor("k", [N, COLS], F32, kind="ExternalInput").ap()
    v = nc.dram_tensor("v", [N, COLS], F32, kind="ExternalInput").ap()
    out = nc.dram_tensor("out", [N, COLS], F32, kind="ExternalOutput").ap()
    with tile.TileContext(nc) as tc:
        if loop:
            with tc.For_i(0, loop, 1):
                _emit(tc, nc, q, k, v, out, **emit_kw)
        else:
            _emit(tc, nc, q, k, v, out, **emit_kw)
    nc.compile()
    _cache[key] = nc
    return nc


def _in_maps(q, k, v):
    maps = []
    for c in range(NCORES):
        b, hp = divmod(c, 4)
        cs = slice(hp * COLS, (hp + 1) * COLS)
        maps.append({
            "q": np.ascontiguousarray(q[b, :, cs], dtype=np.float32),
            "k": np.ascontiguousarray(k[b, :, cs], dtype=np.float32),
            "v": np.ascontiguousarray(v[b, :, cs], dtype=np.float32),
        })
    return maps


def _assemble(results):
    out = np.empty((B, N, C), np.float32)
    for c in range(NCORES):
        b, hp = divmod(c, 4)
        out[b, :, hp * COLS:(hp + 1) * COLS] = results[c]["out"]
    return out


def kernel(q, k, v):
    nc = _build()
    res = bass_utils.run_bass_kernel_spmd(
        nc, _in_maps(q, k, v), core_ids=list(range(NCORES))
    )
    return _assemble(res.results)



# revision 56
# speedup vs baseline: 1.0592x; 1.0592x over previous
"""Multi-head attention kernel for Trainium2, sharded over 8 NeuronCores.

Problem: q,k,v [2, 4096, 256], 8 heads of d=32.  b*h = 16 head-instances
are sharded 2-per-core (core c -> batch c//4, head-pair c%4); no
cross-core communication.

Per-core algorithm (n=4096, d=32, 2 heads, fp16 operands / fp32 PSUM):

  phase 0: DMA q/k slabs [4096,64]; DVE casts to fp16 and duplicates the
    64 channels side-by-side -> staged [128,128] row-chunks; fp16
    PE-transpose (1 cyc/row) to kT/qTt [128, n] with the head-pair
    duplicated on partitions 64-127 (d rows: h0 @0-31, h1 @32-63, dup
    @64-127).  The duplicate feeds PE-array row-tiles (64,0)/(96,0).
    PSUM->SBUF copies alternate ScalarE/DVE.  V stored as per-k-chunk
    [128, 33] tiles with a ones-column appended (softmax denominator
    rides the PV matmul for free).

  main loop (q-tiles of 512, groups of 2 k-chunks of 128):
    - S^T for 2 heads x 2 k-chunks as FOUR concurrent 32-row PE tiles
      (tile_position (0,0),(32,0),(64,0),(96,0)) -> two PSUM groups
      [128, 2x512]; 100% PE row utilization (contraction d=32).
    - exp: split across TWO engines per a per-qt schedule:
        ScalarE chunks: one ACTIVATE Exp [128,1024] PSUM->SBUF fp16.
        DVE chunks: single-piece Schraudolph, ONE VectorE op:
          i = round(S * (scale*1024*log2e) + B)   int16  (1 elem/cyc)
        bitcast_fp16(i) = exp(S*scale) * (1 +- 3% sawtooth); the mean
        ratio is folded into B so DVE chunks match ScalarE chunks'
        scale; softmax normalization cancels any shared factor.
    - O^T[33,512] += [V|1]^T P per head, 2-way col-tiled PV matmuls
      ((0,0)/(0,64)) accumulated in one PSUM bank [97, 512]; row 32/96
      collects the denominator via the ones-column.  PV lags S by
      pv_skew=8 chunks so exp latency (esp. the DVE chain) never stalls
      the PE; the backlog drains during the next q-tile's groups.
    - per-q-tile epilogue (PE-transpose back, multiply by reciprocal
      denominator, DMA out) is spread across the next q-tile's groups.
"""

import numpy as np

import concourse.mybir as mybir
import concourse.tile as tile
from concourse import bacc, bass_utils
from concourse.masks import make_identity

_TAIL_FREE = 0       # 0: spread DVE chunks over all groups (a Scalar-only
                     # tail drains the 3-deep ps pool too slowly at the
                     # q-tile boundary and starves both exp engines)
_DVE_PER_QT = 12     # chunks per q-tile computed on the Vector engine
_PV_SKEW = 8         # chunks of S->exp lookahead before PV consumes

B, N, C, H, D = 2, 4096, 256, 8, 32
NCORES = 8
HPC = 2                      # heads per core
COLS = HPC * D               # 64 per-core channel columns
P = 128                      # partitions / k-chunk
QTILE = 512                  # q columns per head per PSUM group
NKC = N // P                 # 32 k-chunks
NQT = N // QTILE             # 8 q-tiles per head
NG = NKC // 2                # 16 k-chunk groups (4-way S tiling)
SCALE = float(1.0 / np.sqrt(D))
F32 = mybir.dt.float32
BF16 = mybir.dt.float16  # fp16: same PE speed as bf16, 4x finer mantissa
I16 = mybir.dt.int16

# Schraudolph pair-trick constants (see docstring).  i = round(A*x + Bc),
# P = fp16_bits(i) + 2^-0.5 * fp16_bits(i+512) approximates
# MEANR * exp(x*SCALE); log2(MEANR) is subtracted from the bias so the
# result matches ScalarE chunks' exp() scale.
LOG2E = float(1.0 / np.log(2.0))
AEXP = float(SCALE * 1024.0 * LOG2E)
MEANR = 2.0813747       # E[pair/exp] at zero bias shift; measured, mean->1
BEXP = float(1024.0 * 15.0 - 1024.0 * np.log2(MEANR))
HALF = float(2.0 ** -0.5)

_cache = {}


def _dve_sched(dve_per_qt, tail_free=_TAIL_FREE):
    """Bresenham spread of DVE-exp chunks over the first NG - tail_free
    groups of a q-tile (the final groups stay on ScalarE so it carries a
    backlog into the q-tile boundary while the PE drains PVs; DVE has
    per-tile slack, so its boundary idle is free).  Returns per-group
    (kcA_dve, kcB_dve); kcA prefers ScalarE."""
    ng = NG - tail_free
    sched = []
    for g in range(NG):
        if g >= ng:
            sched.append((False, False))
            continue
        n0 = (g * dve_per_qt) // ng
        n1 = ((g + 1) * dve_per_qt) // ng
        take = n1 - n0          # 0, 1 (or 2 when dve_per_qt > ng)
        sched.append((take >= 2, take >= 1))
    return sched


def _emit(tc, nc, q, k, v, out, heads=HPC, dve_per_qt=_DVE_PER_QT,
          do_s=True, do_exp=True, do_pv=True, do_main=True,
          pv_skew=_PV_SKEW):
    sched = _dve_sched(dve_per_qt)
    with tc.tile_pool(name="persist", bufs=1) as persist:
        ident = persist.tile([P, P], F32, name="ident")
        make_identity(nc, ident[:])
        # kT: xbar-transposed K with TWO k-chunks stacked per 128-col block:
        # block g partitions 0-63 = chunk 2g (h0 rows 0-31, h1 rows 32-63),
        # partitions 64-127 = chunk 2g+1.  Chunk c holds k rows {32p + c}
        # (a permutation of the natural order; V is gathered to match, and
        # softmax contraction is k-order invariant).
        kT = persist.tile([P, NG * P], BF16, name="kT")
        # qT duplicated layout: partitions 0-63 = (h0 d, h1 d), 64-127 =
        # copy (feeds PE row-tiles (64,0)/(96,0)); q contiguous on columns.
        qT = persist.tile([P, N], BF16, name="qT")
        # V with ones column: per head, 32 chunks of [128, 33].  vbs is
        # the same scaled by 2^-0.5 — the second Schraudolph pair piece's
        # weight, so the pair-sum rides the PV PSUM accumulation.
        vsb = persist.tile([P, HPC * NKC * (D + 1)], BF16, name="vsb")
        vbs = persist.tile([P, HPC * NKC * (D + 1)], BF16, name="vbs")

        identh = persist.tile([P, P], BF16, name="identh")
        nc.vector.tensor_copy(identh[:], ident[:])
        # K: two contiguous fp32 loads (cheap, 128 descriptors each),
        # DVE cast to fp16, xbar block-transpose per half straight into
        # the stacked-chunk kT layout.  Q: gathered per quarter
        # (n-on-partitions); its cast + duplicate + fp16 PE-transposes
        # are emitted LAZILY inside the main loop (just ahead of the
        # q-tiles that need them) so the per-engine FIFOs never park the
        # first S matmuls behind staging that waits on late gathers.
        # V: gathered in the chunk order matching kT's xbar permutation;
        # its SBUF staging is likewise emitted inside q-tile 0.
        NH = 2
        KHF = N * COLS // P // NH        # fp16 elems per partition/half
        NQ = 4
        CPQ = NKC // NQ                  # 8 q-chunks per quarter
        VHC = NKC // NH
        stage_pool = persist
        kf = [stage_pool.tile([P, KHF], F32, name=f"kf{h}")
              for h in range(NH)]
        kst = [stage_pool.tile([P, KHF], BF16, name=f"kst{h}")
               for h in range(NH)]
        qf = [stage_pool.tile([P, CPQ * COLS], F32, name=f"qf{g}")
              for g in range(NQ)]
        stq = [stage_pool.tile([P, CPQ * 2 * COLS], BF16,
                               name=f"stq{g}") for g in range(NQ)]
        vstage = stage_pool.tile([P, NKC * COLS], F32, name="vstage")
        vsv = vstage[:].rearrange("p (c d) -> p c d", d=COLS)
        onescol = persist.tile([P, HPC * NKC], F32, name="onescol")

        qsrc = q.rearrange("(i p) d -> i p d", p=P)

        def qload(g):
            nc.gpsimd.dma_start(
                qf[g][:].rearrange("p (i d) -> p i d", d=COLS),
                qsrc[g * CPQ:(g + 1) * CPQ].rearrange("i p d -> p i d"),
            )

        # Ring split: k halves + q quarter 0 (they gate the first S group)
        # on the HWDGE sync ring; v and q quarters 1-3 on the SWDGE
        # (gpsimd) ring, which drains concurrently — v is needed by the
        # first PV pops (~4 groups in) and must not queue behind q0.
        nc.gpsimd.dma_start(
            kf[0][:], k.rearrange("(h p x) d -> h p (x d)", h=NH, p=P)[0]
        )
        qload(0)
        nc.gpsimd.dma_start(
            kf[1][:], k.rearrange("(h p x) d -> h p (x d)", h=NH, p=P)[1]
        )
        for h in range(NH):
            nc.gpsimd.dma_start(
                vsv[:, h * VHC:(h + 1) * VHC, :],
                v.rearrange("(h p c) d -> h p c d", h=NH, p=P)[h],
            )
        for g in range(1, NQ):
            nc.gpsimd.dma_start(
                qf[g][:].rearrange("p (i d) -> p i d", d=COLS),
                qsrc[g * CPQ:(g + 1) * CPQ].rearrange("i p d -> p i d"),
            )

        # k casts on ScalarE (idle until the first exp; keeps the DVE FIFO
        # free for q quarter 0's cast+dup, which gates the first S group);
        # xbars at the tail of the sync HWDGE queue: FIFO puts them after
        # the three sync loads, and every DMA scheduled after them (only
        # the much-later output stores) inherits their wait.
        for h in range(NH):
            nc.vector.tensor_copy(kst[h][:], kf[h][:])
            nc.sync.dma_start_transpose(
                kT[:, h * KHF:(h + 1) * KHF].rearrange(
                    "p (b c) -> p b c", c=P),
                kst[h][:],
            )

        def stage_v():
            vv = vsb[:].rearrange("p (hh i e) -> p hh i e",
                                  hh=HPC, e=D + 1)
            vst = vstage[:].rearrange("p (i d) -> p i d", d=COLS)
            for hh in range(HPC):
                nc.vector.tensor_copy(
                    vv[:, hh, :, 0:D], vst[:, :, hh * D:(hh + 1) * D]
                )
            nc.vector.memset(onescol[:], 1.0)
            nc.vector.tensor_copy(
                vv[:, :, :, D],
                onescol[:].rearrange("p (hh i) -> p hh i", hh=HPC),
            )
            nc.vector.tensor_scalar_mul(vbs[:], vsb[:], HALF)

        if not do_main:
            return
        with (
            tc.tile_pool(name="ps", bufs=3, space="PSUM") as ps_pool,
            tc.tile_pool(name="po", bufs=2, space="PSUM") as po_pool,
            tc.tile_pool(name="pexp", bufs=pv_skew + 2) as pexp_pool,
            tc.tile_pool(name="pint", bufs=pv_skew // 2 + 2) as pint_pool,
            tc.tile_pool(name="pint2", bufs=pv_skew // 2 + 2) as pint2_pool,
            tc.tile_pool(name="osb", bufs=2) as osb_pool,
            tc.tile_pool(name="rec", bufs=3) as rec_pool,
            tc.tile_pool(name="outsb", bufs=3) as outsb_pool,
        ):
            if not do_s:
                ps_fix = ps_pool.tile([P, HPC * QTILE], F32, tag="ps")
                nc.vector.memset(ps_fix[:], 0.25)
            if not do_exp:
                pexp_fix = pexp_pool.tile([P, HPC * QTILE], BF16, tag="pexp")
                nc.vector.memset(pexp_fix[:], 0.5)

            def stage_quarter(g):
                """Cast + duplicate + PE-transpose q quarter g into qT
                (borrows ps_pool slots for the transpose PSUM tiles)."""
                sv = stq[g][:].rearrange("p (i two d) -> p i two d",
                                         two=2, d=COLS)
                qv = qf[g][:].rearrange("p (i d) -> p i d", d=COLS)
                nc.vector.tensor_copy(sv[:, :, 0, :], qv)
                nc.vector.tensor_copy(sv[:, :, 1, :], sv[:, :, 0, :])
                for j in range(CPQ // 4):
                    pt = ps_pool.tile([P, 4 * P], BF16, tag="ps",
                                      name=f"pt{g}_{j}", uniquify=True)
                    for jj in range(4):
                        ii = 4 * j + jj
                        nc.tensor.transpose(
                            pt[:, jj * P:(jj + 1) * P],
                            stq[g][:, ii * 2 * COLS:(ii + 1) * 2 * COLS],
                            identh[:],
                        )
                    dst = qT[:, (2 * g + j) * 512:(2 * g + j + 1) * 512]
                    nc.vector.tensor_copy(dst, pt[:])

            stage_quarter(0)

            def emit_s_half(qt, g, half):
                """One half of the 4-way row-tiled S^T group: half 0 ->
                chunk 2g on PE rows 0-63 (kT block g partitions 0-63),
                half 1 -> chunk 2g+1 on rows 64-127."""
                if not do_s:
                    return ps_fix
                ps = ps_pool.tile([P, HPC * QTILE], F32, tag="ps")
                base = 64 * half
                for hh in range(heads):
                    rp = slice(base + D * hh, base + D * (hh + 1))
                    nc.tensor.matmul(
                        ps[:, hh * QTILE:(hh + 1) * QTILE],
                        lhsT=kT[rp, g * P:(g + 1) * P],
                        rhs=qT[rp, qt * QTILE:(qt + 1) * QTILE],
                        start=True, stop=True,
                        tile_position=(base + D * hh, 0),
                    )
                return ps

            def emit_exp(ps, use_dve):
                """exp of one [128, 2*512] S chunk.  ScalarE: one fp16 tile.
                DVE: two int16 Schraudolph pieces (summed later on the PE).
                The fp16 S source gives the DVE op 2x_1P packed reads."""
                if not do_exp:
                    return ((pexp_fix,), False)
                if not use_dve:
                    pexp = pexp_pool.tile([P, HPC * QTILE], BF16, tag="pexp")
                    nc.scalar.activation(
                        pexp[:], ps[:], mybir.ActivationFunctionType.Exp,
                        scale=SCALE,
                    )
                    return ((pexp,), False)
                pint = pint_pool.tile([P, HPC * QTILE], I16, tag="pint")
                nc.vector.tensor_scalar(
                    pint[:], ps[:], AEXP, BEXP,
                    mybir.AluOpType.mult, mybir.AluOpType.add,
                )
                pint2 = pint2_pool.tile([P, HPC * QTILE], I16, tag="pint2")
                nc.vector.tensor_scalar_add(pint2[:], pint[:], 512)
                return ((pint, pint2), True)

            def emit_pv(poc, pieces, kc, first, last):
                if not do_pv:
                    return
                tiles, isint = pieces
                vws = (vsb, vbs)[:len(tiles)]
                for pi, (tl, vw) in enumerate(zip(tiles, vws)):
                    st = first and pi == 0
                    sp = last and pi == len(tiles) - 1
                    for hh in range(heads):
                        vbase = hh * NKC * (D + 1)
                        vch = vw[:, vbase + kc * (D + 1):
                                 vbase + (kc + 1) * (D + 1)]
                        rhs = tl[:, hh * QTILE:(hh + 1) * QTILE]
                        if isint:
                            rhs = rhs.bitcast(BF16)
                        nc.tensor.matmul(
                            poc[64 * hh:64 * hh + D + 1, :],
                            lhsT=vch,
                            rhs=rhs,
                            start=st,
                            stop=sp,
                            skip_group_check=True,
                            tile_position=(0, 64 * hh),
                        )

            def epilogue_steps(pocl, q0):
                def copy_step():
                    osb = osb_pool.tile([97, QTILE], F32, tag="osb",
                                        name="osb", uniquify=True)
                    osbs[0] = osb
                    for hh in range(heads):
                        ib = 64 * hh
                        nc.vector.tensor_copy(
                            osb[ib:ib + D + 1, :], pocl[ib:ib + D + 1, :]
                        )
                def norm_step(j):
                    # both heads -> one [128, 64] tile -> one contiguous
                    # DMA; both transposes share ONE ps-pool slot so the
                    # epilogue never starves S of PSUM buffers
                    outsb = outsb_pool.tile([P, COLS], F32, tag="outsb")
                    pt2 = ps_pool.tile([P, 2 * (D + 1)], F32, tag="ps",
                                       name="pt2", uniquify=True)
                    for hh in range(heads):
                        ib = 64 * hh
                        ob = (D + 1) * hh
                        nc.tensor.transpose(
                            pt2[:, ob:ob + D + 1],
                            osbs[0][ib:ib + D + 1, j * P:(j + 1) * P],
                            ident[ib:ib + D + 1, ib:ib + D + 1],
                        )
                        rec = rec_pool.tile([P, 1], F32, tag="rec")
                        nc.vector.reciprocal(
                            rec[:], pt2[:, ob + D:ob + D + 1]
                        )
                        nc.vector.tensor_scalar_mul(
                            outsb[:, D * hh:D * (hh + 1)],
                            pt2[:, ob:ob + D], rec[:],
                        )
                    nc.sync.dma_start(
                        out[q0 + j * P:q0 + (j + 1) * P, :], outsb[:]
                    )
                osbs = {}
                steps = [copy_step]
                for j in range(QTILE // P):
                    steps.append(lambda j=j: norm_step(j))
                return steps

            pending = []          # deferred epilogue of the previous q-tile
            pvq = []              # (pieces, kc, poc, q0) across q-tiles

            def pop_pv():
                pieces, kc, poc, q0 = pvq.pop(0)
                emit_pv(poc, pieces, kc, kc == 0, kc == NKC - 1)
                if kc == NKC - 1:
                    # this q-tile's accumulation is complete; queue its
                    # epilogue to be spread over the following groups
                    pending.extend(epilogue_steps(poc, q0))

            for qt in range(NQT):
                q0 = qt * QTILE
                if qt % 2 == 1 and qt < NQT - 1:
                    stage_quarter((qt + 1) // 2)
                poc = po_pool.tile([97, QTILE], F32, tag="po",
                                   name=f"po_{qt}")
                for g in range(NG):
                    if qt == 0 and g == 2:
                        stage_v()
                    psA = emit_s_half(qt, g, 0)
                    psB = emit_s_half(qt, g, 1)
                    dveA, dveB = sched[g]
                    pexpA = emit_exp(psA, dveA)
                    pexpB = emit_exp(psB, dveB)
                    # spread the 5 epilogue steps across the whole q-tile
                    # (one per 3 groups) instead of bursting them into the
                    # already PE-heavy boundary groups
                    if pending and g % 3 == 0 and g > 0:
                        pending.pop(0)()
                    pvq.append((pexpA, 2 * g, poc, q0))
                    pvq.append((pexpB, 2 * g + 1, poc, q0))
                    # taper the skew at the very end so the tail drains
                    # alongside the last groups instead of after them
                    thr = pv_skew
                    if qt == NQT - 1:
                        thr = min(pv_skew, 2 * (NG - 1 - g))
                    while len(pvq) > thr:
                        pop_pv()
            while pvq:
                pop_pv()
            for step in pending:
                step()


def _build(loop=0, **emit_kw):
    """loop=0: production build.  loop>=1: body wrapped in an on-device
    For_i repeat loop (timing-only builds).  emit_kw: ablation knobs."""
    key = ("nc", loop, tuple(sorted(emit_kw.items())))
    if key in _cache:
        return _cache[key]
    nc = bacc.Bacc(
        "TRN2",
        target_bir_lowering=False,
        debug=False,
        enable_asserts=False,
        num_devices=NCORES,
    )
    q = nc.dram_tensor("q", [N, COLS], F32, kind="ExternalInput").ap()
    k = nc.dram_tensor("k", [N, COLS], F32, kind="ExternalInput").ap()
    v = nc.dram_tensor("v", [N, COLS], F32, kind="ExternalInput").ap()
    out = nc.dram_tensor("out", [N, COLS], F32, kind="ExternalOutput").ap()
    with tile.TileContext(nc) as tc:
        if loop:
            with tc.For_i(0, loop, 1):
                _emit(tc, nc, q, k, v, out, **emit_kw)
        else:
            _emit(tc, nc, q, k, v, out, **emit_kw)
    nc.compile()
    _cache[key] = nc
    return nc


def _in_maps(q, k, v):
    maps = []
    for c in range(NCORES):
        b, hp = divmod(c, 4)
        cs = slice(hp * COLS, (hp + 1) * COLS)
        maps.append({
            "q": np.ascontiguousarray(q[b, :, cs], dtype=np.float32),
            "k": np.ascontiguousarray(k[b, :, cs], dtype=np.float32),
            "v": np.ascontiguousarray(v[b, :, cs], dtype=np.float32),
        })
    return maps


def _assemble(results):
    out = np.empty((B, N, C), np.float32)
    for c in range(NCORES):
        b, hp = divmod(c, 4)
        out[b, :, hp * COLS:(hp + 1) * COLS] = results[c]["out"]
    return out


def kernel(q, k, v):
    nc = _build()
    res = bass_utils.run_bass_kernel_spmd(
        nc, _in_maps(q, k, v), core_ids=list(range(NCORES))
    )
    return _assemble(res.results)



# revision 57
# speedup vs baseline: 1.0610x; 1.0017x over previous
"""Multi-head attention kernel for Trainium2, sharded over 8 NeuronCores.

Problem: q,k,v [2, 4096, 256], 8 heads of d=32.  b*h = 16 head-instances
are sharded 2-per-core (core c -> batch c//4, head-pair c%4); no
cross-core communication.

Per-core algorithm (n=4096, d=32, 2 heads, fp16 operands / fp32 PSUM):

  staging (mostly lazy, interleaved with the main loop):
    - K loads CONTIGUOUSLY (fp32, 2 cheap halves), DVE-casts to fp16,
      then ONE xbar DMA-transpose per half lands kT [128, 16x128] with
      TWO k-chunks stacked per block (partitions 0-63 = chunk 2g,
      64-127 = chunk 2g+1) — exactly the 4-band S weight layout, no PE
      transposes and no duplicate copy.  The chunk order is a
      permutation (chunk c = k rows {2048*(c//16) + 16p + c%16}); V is
      gathered to match and softmax is k-order invariant.
    - Q gathers per quarter (n-on-partitions) + DVE cast/dup + fp16
      PE-transposes into qT (dup layout).  Quarters 1-3 are emitted
      INSIDE the main loop just before the q-tiles that need them, so
      the PE/DVE FIFOs never park the first S matmuls behind staging
      that waits on late gathers.  V staging likewise lands in q-tile 0.
    - v + q1-3 gathers ride the SWDGE ring, k/q0 + xbars the sync HWDGE
      ring (every DMA scheduled after an xbar waits for it, so xbars
      sit at the tail of the queue).

  main loop (q-tiles of 512, groups of 2 k-chunks of 128):
    - S^T for 2 heads x 2 k-chunks as FOUR concurrent 32-row PE tiles
      (tile_position (0,0),(32,0),(64,0),(96,0)) -> two PSUM groups
      [128, 2x512]; 100% PE row utilization (contraction d=32).
    - exp: split across TWO engines per a per-qt schedule (DVE chunks
      spread over ALL groups — a Scalar-only tail drains the 3-deep ps
      pool too slowly at the q-tile boundary and starves both engines):
        ScalarE chunks: one ACTIVATE Exp [128,1024] PSUM->SBUF fp16.
        DVE chunks: Schraudolph pair-trick, two VectorE ops:
          i  = round(S * (scale*1024*log2e) + B)   int16  (1 elem/cyc)
          i2 = i + 512                                     (4x mode)
        The pair sum bitcast_fp16(i) + 2^-0.5*bitcast_fp16(i2)
        = exp(S*scale) * (1 +- 0.8%) is NOT computed on DVE — it rides
        the PE: the PV step issues one extra matmul pair against vbs
        (= V * 2^-0.5), and the PSUM accumulator adds the pieces.  The
        mean ratio is folded into B so DVE chunks match ScalarE chunks'
        scale; softmax normalization cancels any shared factor.
    - O^T[33,512] += [V|1]^T P per head, 2-way col-tiled PV matmuls
      ((0,0)/(0,64)) accumulated in one PSUM bank [97, 512]; row 32/96
      collects the denominator via the ones-column.  PV lags S by
      pv_skew=8 chunks so exp latency (esp. the DVE chain) never stalls
      the PE; the backlog drains during the next q-tile's groups.
    - per-q-tile epilogue (PE-transpose back — both heads sharing one
      ps-pool slot so the epilogue never starves S of PSUM buffers —
      multiply by reciprocal denominator, DMA out) is spread one step
      per 3 groups across the next q-tile.
"""

import numpy as np

import concourse.mybir as mybir
import concourse.tile as tile
from concourse import bacc, bass_utils
from concourse.masks import make_identity

_TAIL_FREE = 0       # 0: spread DVE chunks over all groups (a Scalar-only
                     # tail drains the 3-deep ps pool too slowly at the
                     # q-tile boundary and starves both exp engines)
_DVE_PER_QT = 12     # chunks per q-tile computed on the Vector engine
_PV_SKEW = 8         # chunks of S->exp lookahead before PV consumes

B, N, C, H, D = 2, 4096, 256, 8, 32
NCORES = 8
HPC = 2                      # heads per core
COLS = HPC * D               # 64 per-core channel columns
P = 128                      # partitions / k-chunk
QTILE = 512                  # q columns per head per PSUM group
NKC = N // P                 # 32 k-chunks
NQT = N // QTILE             # 8 q-tiles per head
NG = NKC // 2                # 16 k-chunk groups (4-way S tiling)
SCALE = float(1.0 / np.sqrt(D))
F32 = mybir.dt.float32
BF16 = mybir.dt.float16  # fp16: same PE speed as bf16, 4x finer mantissa
I16 = mybir.dt.int16

# Schraudolph pair-trick constants (see docstring).  i = round(A*x + Bc),
# P = fp16_bits(i) + 2^-0.5 * fp16_bits(i+512) approximates
# MEANR * exp(x*SCALE); log2(MEANR) is subtracted from the bias so the
# result matches ScalarE chunks' exp() scale.
LOG2E = float(1.0 / np.log(2.0))
AEXP = float(SCALE * 1024.0 * LOG2E)
MEANR = 2.0813747       # E[pair/exp] at zero bias shift; measured, mean->1
BEXP = float(1024.0 * 15.0 - 1024.0 * np.log2(MEANR))
HALF = float(2.0 ** -0.5)

_cache = {}


def _dve_sched(dve_per_qt, tail_free=_TAIL_FREE):
    """Bresenham spread of DVE-exp chunks over the first NG - tail_free
    groups of a q-tile (the final groups stay on ScalarE so it carries a
    backlog into the q-tile boundary while the PE drains PVs; DVE has
    per-tile slack, so its boundary idle is free).  Returns per-group
    (kcA_dve, kcB_dve); kcA prefers ScalarE."""
    ng = NG - tail_free
    sched = []
    for g in range(NG):
        if g >= ng:
            sched.append((False, False))
            continue
        n0 = (g * dve_per_qt) // ng
        n1 = ((g + 1) * dve_per_qt) // ng
        take = n1 - n0          # 0, 1 (or 2 when dve_per_qt > ng)
        sched.append((take >= 2, take >= 1))
    return sched


def _emit(tc, nc, q, k, v, out, heads=HPC, dve_per_qt=_DVE_PER_QT,
          do_s=True, do_exp=True, do_pv=True, do_main=True,
          pv_skew=_PV_SKEW):
    sched = _dve_sched(dve_per_qt)
    with tc.tile_pool(name="persist", bufs=1) as persist:
        ident = persist.tile([P, P], F32, name="ident")
        make_identity(nc, ident[:])
        # kT: xbar-transposed K with TWO k-chunks stacked per 128-col block:
        # block g partitions 0-63 = chunk 2g (h0 rows 0-31, h1 rows 32-63),
        # partitions 64-127 = chunk 2g+1.  Chunk c holds k rows {32p + c}
        # (a permutation of the natural order; V is gathered to match, and
        # softmax contraction is k-order invariant).
        kT = persist.tile([P, NG * P], BF16, name="kT")
        # qT duplicated layout: partitions 0-63 = (h0 d, h1 d), 64-127 =
        # copy (feeds PE row-tiles (64,0)/(96,0)); q contiguous on columns.
        qT = persist.tile([P, N], BF16, name="qT")
        # V with ones column: per head, 32 chunks of [128, 33].  vbs is
        # the same scaled by 2^-0.5 — the second Schraudolph pair piece's
        # weight, so the pair-sum rides the PV PSUM accumulation.
        vsb = persist.tile([P, HPC * NKC * (D + 1)], BF16, name="vsb")
        vbs = persist.tile([P, HPC * NKC * (D + 1)], BF16, name="vbs")

        identh = persist.tile([P, P], BF16, name="identh")
        nc.vector.tensor_copy(identh[:], ident[:])
        # K: two contiguous fp32 loads (cheap, 128 descriptors each),
        # DVE cast to fp16, xbar block-transpose per half straight into
        # the stacked-chunk kT layout.  Q: gathered per quarter
        # (n-on-partitions); its cast + duplicate + fp16 PE-transposes
        # are emitted LAZILY inside the main loop (just ahead of the
        # q-tiles that need them) so the per-engine FIFOs never park the
        # first S matmuls behind staging that waits on late gathers.
        # V: gathered in the chunk order matching kT's xbar permutation;
        # its SBUF staging is likewise emitted inside q-tile 0.
        NH = 2
        KHF = N * COLS // P // NH        # fp16 elems per partition/half
        NQ = 4
        CPQ = NKC // NQ                  # 8 q-chunks per quarter
        VHC = NKC // NH
        stage_pool = persist
        kf = [stage_pool.tile([P, KHF], F32, name=f"kf{h}")
              for h in range(NH)]
        kst = [stage_pool.tile([P, KHF], BF16, name=f"kst{h}")
               for h in range(NH)]
        qf = [stage_pool.tile([P, CPQ * COLS], F32, name=f"qf{g}")
              for g in range(NQ)]
        stq = [stage_pool.tile([P, CPQ * 2 * COLS], BF16,
                               name=f"stq{g}") for g in range(NQ)]
        vstage = stage_pool.tile([P, NKC * COLS], F32, name="vstage")
        vsv = vstage[:].rearrange("p (c d) -> p c d", d=COLS)
        onescol = persist.tile([P, HPC * NKC], F32, name="onescol")

        qsrc = q.rearrange("(i p) d -> i p d", p=P)

        def qload(g):
            nc.gpsimd.dma_start(
                qf[g][:].rearrange("p (i d) -> p i d", d=COLS),
                qsrc[g * CPQ:(g + 1) * CPQ].rearrange("i p d -> p i d"),
            )

        # Ring split: k halves + q quarter 0 (they gate the first S group)
        # on the HWDGE sync ring; v and q quarters 1-3 on the SWDGE
        # (gpsimd) ring, which drains concurrently — v is needed by the
        # first PV pops (~4 groups in) and must not queue behind q0.
        nc.gpsimd.dma_start(
            kf[0][:], k.rearrange("(h p x) d -> h p (x d)", h=NH, p=P)[0]
        )
        qload(0)
        nc.gpsimd.dma_start(
            kf[1][:], k.rearrange("(h p x) d -> h p (x d)", h=NH, p=P)[1]
        )
        for h in range(NH):
            nc.gpsimd.dma_start(
                vsv[:, h * VHC:(h + 1) * VHC, :],
                v.rearrange("(h p c) d -> h p c d", h=NH, p=P)[h],
            )
        for g in range(1, NQ):
            nc.gpsimd.dma_start(
                qf[g][:].rearrange("p (i d) -> p i d", d=COLS),
                qsrc[g * CPQ:(g + 1) * CPQ].rearrange("i p d -> p i d"),
            )

        # k casts on ScalarE (idle until the first exp; keeps the DVE FIFO
        # free for q quarter 0's cast+dup, which gates the first S group);
        # xbars at the tail of the sync HWDGE queue: FIFO puts them after
        # the three sync loads, and every DMA scheduled after them (only
        # the much-later output stores) inherits their wait.
        for h in range(NH):
            nc.vector.tensor_copy(kst[h][:], kf[h][:])
            nc.sync.dma_start_transpose(
                kT[:, h * KHF:(h + 1) * KHF].rearrange(
                    "p (b c) -> p b c", c=P),
                kst[h][:],
            )

        def stage_v():
            vv = vsb[:].rearrange("p (hh i e) -> p hh i e",
                                  hh=HPC, e=D + 1)
            vst = vstage[:].rearrange("p (i d) -> p i d", d=COLS)
            for hh in range(HPC):
                nc.vector.tensor_copy(
                    vv[:, hh, :, 0:D], vst[:, :, hh * D:(hh + 1) * D]
                )
            nc.vector.memset(onescol[:], 1.0)
            nc.vector.tensor_copy(
                vv[:, :, :, D],
                onescol[:].rearrange("p (hh i) -> p hh i", hh=HPC),
            )
            nc.vector.tensor_scalar_mul(vbs[:], vsb[:], HALF)

        if not do_main:
            return
        with (
            tc.tile_pool(name="ps", bufs=3, space="PSUM") as ps_pool,
            tc.tile_pool(name="po", bufs=2, space="PSUM") as po_pool,
            tc.tile_pool(name="pexp", bufs=pv_skew + 2) as pexp_pool,
            tc.tile_pool(name="pint", bufs=pv_skew // 2 + 2) as pint_pool,
            tc.tile_pool(name="pint2", bufs=pv_skew // 2 + 2) as pint2_pool,
            tc.tile_pool(name="osb", bufs=2) as osb_pool,
            tc.tile_pool(name="rec", bufs=3) as rec_pool,
            tc.tile_pool(name="outsb", bufs=3) as outsb_pool,
        ):
            if not do_s:
                ps_fix = ps_pool.tile([P, HPC * QTILE], F32, tag="ps")
                nc.vector.memset(ps_fix[:], 0.25)
            if not do_exp:
                pexp_fix = pexp_pool.tile([P, HPC * QTILE], BF16, tag="pexp")
                nc.vector.memset(pexp_fix[:], 0.5)

            def stage_quarter(g):
                """Cast + duplicate + PE-transpose q quarter g into qT
                (borrows ps_pool slots for the transpose PSUM tiles)."""
                sv = stq[g][:].rearrange("p (i two d) -> p i two d",
                                         two=2, d=COLS)
                qv = qf[g][:].rearrange("p (i d) -> p i d", d=COLS)
                nc.vector.tensor_copy(sv[:, :, 0, :], qv)
                nc.vector.tensor_copy(sv[:, :, 1, :], sv[:, :, 0, :])
                for j in range(CPQ // 4):
                    pt = ps_pool.tile([P, 4 * P], BF16, tag="ps",
                                      name=f"pt{g}_{j}", uniquify=True)
                    for jj in range(4):
                        ii = 4 * j + jj
                        nc.tensor.transpose(
                            pt[:, jj * P:(jj + 1) * P],
                            stq[g][:, ii * 2 * COLS:(ii + 1) * 2 * COLS],
                            identh[:],
                        )
                    dst = qT[:, (2 * g + j) * 512:(2 * g + j + 1) * 512]
                    nc.vector.tensor_copy(dst, pt[:])

            stage_quarter(0)

            def emit_s_half(qt, g, half):
                """One half of the 4-way row-tiled S^T group: half 0 ->
                chunk 2g on PE rows 0-63 (kT block g partitions 0-63),
                half 1 -> chunk 2g+1 on rows 64-127."""
                if not do_s:
                    return ps_fix
                ps = ps_pool.tile([P, HPC * QTILE], F32, tag="ps")
                base = 64 * half
                for hh in range(heads):
                    rp = slice(base + D * hh, base + D * (hh + 1))
                    nc.tensor.matmul(
                        ps[:, hh * QTILE:(hh + 1) * QTILE],
                        lhsT=kT[rp, g * P:(g + 1) * P],
                        rhs=qT[rp, qt * QTILE:(qt + 1) * QTILE],
                        start=True, stop=True,
                        tile_position=(base + D * hh, 0),
                    )
                return ps

            def emit_exp(ps, use_dve):
                """exp of one [128, 2*512] S chunk.  ScalarE: one fp16 tile.
                DVE: two int16 Schraudolph pieces (summed later on the PE).
                The fp16 S source gives the DVE op 2x_1P packed reads."""
                if not do_exp:
                    return ((pexp_fix,), False)
                if not use_dve:
                    pexp = pexp_pool.tile([P, HPC * QTILE], BF16, tag="pexp")
                    nc.scalar.activation(
                        pexp[:], ps[:], mybir.ActivationFunctionType.Exp,
                        scale=SCALE,
                    )
                    return ((pexp,), False)
                pint = pint_pool.tile([P, HPC * QTILE], I16, tag="pint")
                nc.vector.tensor_scalar(
                    pint[:], ps[:], AEXP, BEXP,
                    mybir.AluOpType.mult, mybir.AluOpType.add,
                )
                pint2 = pint2_pool.tile([P, HPC * QTILE], I16, tag="pint2")
                nc.vector.tensor_scalar_add(pint2[:], pint[:], 512)
                return ((pint, pint2), True)

            def emit_pv(poc, pieces, kc, first, last):
                if not do_pv:
                    return
                tiles, isint = pieces
                vws = (vsb, vbs)[:len(tiles)]
                for pi, (tl, vw) in enumerate(zip(tiles, vws)):
                    st = first and pi == 0
                    sp = last and pi == len(tiles) - 1
                    for hh in range(heads):
                        vbase = hh * NKC * (D + 1)
                        vch = vw[:, vbase + kc * (D + 1):
                                 vbase + (kc + 1) * (D + 1)]
                        rhs = tl[:, hh * QTILE:(hh + 1) * QTILE]
                        if isint:
                            rhs = rhs.bitcast(BF16)
                        nc.tensor.matmul(
                            poc[64 * hh:64 * hh + D + 1, :],
                            lhsT=vch,
                            rhs=rhs,
                            start=st,
                            stop=sp,
                            skip_group_check=True,
                            tile_position=(0, 64 * hh),
                        )

            def epilogue_steps(pocl, q0):
                def copy_step():
                    osb = osb_pool.tile([97, QTILE], F32, tag="osb",
                                        name="osb", uniquify=True)
                    osbs[0] = osb
                    for hh in range(heads):
                        ib = 64 * hh
                        nc.vector.tensor_copy(
                            osb[ib:ib + D + 1, :], pocl[ib:ib + D + 1, :]
                        )
                def norm_step(j):
                    # both heads -> one [128, 64] tile -> one contiguous
                    # DMA; both transposes share ONE ps-pool slot so the
                    # epilogue never starves S of PSUM buffers
                    outsb = outsb_pool.tile([P, COLS], F32, tag="outsb")
                    pt2 = ps_pool.tile([P, 2 * (D + 1)], F32, tag="ps",
                                       name="pt2", uniquify=True)
                    for hh in range(heads):
                        ib = 64 * hh
                        ob = (D + 1) * hh
                        nc.tensor.transpose(
                            pt2[:, ob:ob + D + 1],
                            osbs[0][ib:ib + D + 1, j * P:(j + 1) * P],
                            ident[ib:ib + D + 1, ib:ib + D + 1],
                        )
                        rec = rec_pool.tile([P, 1], F32, tag="rec")
                        nc.vector.reciprocal(
                            rec[:], pt2[:, ob + D:ob + D + 1]
                        )
                        nc.vector.tensor_scalar_mul(
                            outsb[:, D * hh:D * (hh + 1)],
                            pt2[:, ob:ob + D], rec[:],
                        )
                    nc.sync.dma_start(
                        out[q0 + j * P:q0 + (j + 1) * P, :], outsb[:]
                    )
                osbs = {}
                steps = [copy_step]
                for j in range(QTILE // P):
                    steps.append(lambda j=j: norm_step(j))
                return steps

            pending = []          # deferred epilogue of the previous q-tile
            pvq = []              # (pieces, kc, poc, q0) across q-tiles

            def pop_pv():
                pieces, kc, poc, q0 = pvq.pop(0)
                emit_pv(poc, pieces, kc, kc == 0, kc == NKC - 1)
                if kc == NKC - 1:
                    # this q-tile's accumulation is complete; queue its
                    # epilogue to be spread over the following groups
                    pending.extend(epilogue_steps(poc, q0))

            for qt in range(NQT):
                q0 = qt * QTILE
                if qt % 2 == 1 and qt < NQT - 1:
                    stage_quarter((qt + 1) // 2)
                poc = po_pool.tile([97, QTILE], F32, tag="po",
                                   name=f"po_{qt}")
                for g in range(NG):
                    if qt == 0 and g == 2:
                        stage_v()
                    psA = emit_s_half(qt, g, 0)
                    psB = emit_s_half(qt, g, 1)
                    dveA, dveB = sched[g]
                    pexpA = emit_exp(psA, dveA)
                    pexpB = emit_exp(psB, dveB)
                    # spread the 5 epilogue steps across the whole q-tile
                    # (one per 3 groups) instead of bursting them into the
                    # already PE-heavy boundary groups
                    if pending and g % 3 == 0 and g > 0:
                        pending.pop(0)()
                    pvq.append((pexpA, 2 * g, poc, q0))
                    pvq.append((pexpB, 2 * g + 1, poc, q0))
                    # taper the skew at the very end so the tail drains
                    # alongside the last groups instead of after them
                    thr = pv_skew
                    if qt == NQT - 1:
                        thr = min(pv_skew, 2 * (NG - 1 - g))
                    while len(pvq) > thr:
                        pop_pv()
            while pvq:
                pop_pv()
            for step in pending:
                step()


def _build(loop=0, **emit_kw):
    """loop=0: production build.  loop>=1: body wrapped in an on-device
    For_i repeat loop (timing-only builds).  emit_kw: ablation knobs."""
    key = ("nc", loop, tuple(sorted(emit_kw.items())))
    if key in _cache:
        return _cache[key]
    nc = bacc.Bacc(
        "TRN2",
        target_bir_lowering=False,
        debug=False,
        enable_asserts=False,
        num_devices=NCORES,
    )
    q = nc.dram_tensor("q", [N, COLS], F32, kind="ExternalInput").ap()
    k = nc.dram_tensor("k", [N, COLS], F32, kind="ExternalInput").ap()
    v = nc.dram_tensor("v", [N, COLS], F32, kind="ExternalInput").ap()
    out = nc.dram_tensor("out", [N, COLS], F32, kind="ExternalOutput").ap()
    with tile.TileContext(nc) as tc:
        if loop:
            with tc.For_i(0, loop, 1):
                _emit(tc, nc, q, k, v, out, **emit_kw)
        else:
            _emit(tc, nc, q, k, v, out, **emit_kw)
    nc.compile()
    _cache[key] = nc
    return nc


def _in_maps(q, k, v):
    maps = []
    for c in range(NCORES):
        b, hp = divmod(c, 4)
        cs = slice(hp * COLS, (hp + 1) * COLS)
        maps.append({
            "q": np.ascontiguousarray(q[b, :, cs], dtype=np.float32),
            "k": np.ascontiguousarray(k[b, :, cs], dtype=np.float32),
            "v": np.ascontiguousarray(v[b, :, cs], dtype=np.float32),
        })
    return maps


def _assemble(results):
    out = np.empty((B, N, C), np.float32)
    for c in range(NCORES):
        b, hp = divmod(c, 4)
        out[b, :, hp * COLS:(hp + 1) * COLS] = results[c]["out"]
    return out


def kernel(q, k, v):
    nc = _build()
    res = bass_utils.run_bass_kernel_spmd(
        nc, _in_maps(q, k, v), core_ids=list(range(NCORES))
    )
    return _assemble(res.results)

